# revision 20
# baseline (speedup 1.0000x reference)
"""Trainium2 Bass kernel for ConvolutionalAttention2D (linear attention with 1x1 convs).

Reference computation (per batch b):
    q = Wq x ; k = Wk x ; v = Wv x          (1x1 convs == channel matmuls)
    phi(t) = elu(t) + 1
    qv = phi(q) @ phi(v)^T                  ([C, C] context matrix, contract over pixels)
    out = Wo (qv @ phi(k)) + bo

Kernel strategy (8 NeuronCores, data-parallel over batch B=16 -> 2 batches/core):
  - Weights replicated, passed pre-transposed from host.
  - Algebraic refactor: Wo (qv @ phi_k) == (Wo qv) @ phi_k; Wo qv is a tiny
    [C, C] product, saving a full [C, HW] projection matmul per batch.
  - phi(t) = min(exp(t), max(t+1, 1)) computed with 1 ACT pass (Exp) and 2 DVE
    ops (or 2 ACT + 1 DVE, mixed to balance engine load).
  - Projections run on the PE in float32r (full fp32 data, 1 cycle/row);
    attention matmuls in bf16 (phi outputs).
"""

from contextlib import ExitStack

import numpy as np

import concourse.bacc as bacc
import concourse.tile as tile
from concourse import mybir
from concourse import bass_utils

B, C, H, W = 16, 256, 64, 64
HW = H * W
NCORES = 8
NB = B // NCORES  # batches per core

FP = mybir.dt.float32
BF = mybir.dt.bfloat16
F32R = mybir.dt.float32r
AF = mybir.ActivationFunctionType
OP = mybir.AluOpType


def flat2(ap):
    return ap.rearrange("p a b -> p (a b)")


def build_kernel(repeat: int = 1, xp_bufs=3, phikp_bufs=2, pqvp_bufs=1, mm_bufs=3,
                 tmps_bufs=4, outp_bufs=6, schemeb_mod=3):
    """Build the per-core Bass program. `repeat` wraps the whole body in a
    dynamic For_i loop (used only for wall-clock timing runs)."""
    nc = bacc.Bacc("TRN2", target_bir_lowering=False, debug=False)

    x_d = nc.dram_tensor("x", [NB, C, HW], F32R, kind="ExternalInput")
    # all four transposed weights in one tensor: [w, cc, 128, C]
    w_d = nc.dram_tensor("wall", [4, 2, 128, C], F32R, kind="ExternalInput")
    bo_d = nc.dram_tensor("bo", [C, 1], FP, kind="ExternalInput")
    out_d = nc.dram_tensor("out", [NB, C, HW], FP, kind="ExternalOutput")

    with tile.TileContext(nc) as tc, ExitStack() as ctx:
        singles = ctx.enter_context(tc.tile_pool(name="singles", bufs=1))
        xp = ctx.enter_context(tc.tile_pool(name="xp", bufs=xp_bufs))
        phikp = ctx.enter_context(tc.tile_pool(name="phikp", bufs=phikp_bufs))
        pqvp = ctx.enter_context(tc.tile_pool(name="pqvp", bufs=pqvp_bufs))
        tmps = ctx.enter_context(tc.tile_pool(name="tmps", bufs=tmps_bufs))
        smalls = ctx.enter_context(tc.tile_pool(name="smalls", bufs=2))
        outp = ctx.enter_context(tc.tile_pool(name="outp", bufs=outp_bufs))
        psmm = ctx.enter_context(tc.tile_pool(name="psmm", bufs=mm_bufs, space="PSUM"))
        psacc = psmm if mm_bufs >= 4 else ctx.enter_context(tc.tile_pool(name="psacc", bufs=1, space="PSUM"))

        # ---- weights (loaded once, replicated; stage-B weights first) ----
        w_all = singles.tile([128, 4, 2, C], F32R, tag="wall")
        nc.sync.dma_start(
            out=w_all[:, 0:2],
            in_=w_d.ap()[0:2].rearrange("w cc p b -> p w cc b"),
        )
        w_sb = {}
        for wi, name in enumerate(("wqt", "wvt", "wkt", "wot")):
            for cc in range(2):
                w_sb[(name, cc)] = w_all[:, wi, cc, :]
        bo_sb = singles.tile([128, 2], FP, tag="bo")
        for m in range(2):
            nc.sync.dma_start(
                out=bo_sb[:, m:m + 1], in_=bo_d.ap()[m * 128:(m + 1) * 128, :]
            )

        state = {"span": 0, "out": 0}

        def phi_span(psum_ap, dst_ap):
            """dst = phi(psum) = min(exp(x), max(x+1, 1)), bf16 out.

            Scheme A (1 ACT + 2 DVE): e=Exp(x); t=min(e,1); dst=(x max 0)+t
            Scheme B (2 ACT + 1 DVE): e=Exp(x); r=Relu(x); dst=(e min 1)+r
            Mixed by span index to balance ACT vs DVE load.
            """
            i = state["span"]
            state["span"] += 1
            e = tmps.tile([128, 1024], BF, tag="e")
            nc.scalar.activation(e[:], psum_ap, AF.Exp)
            if i % schemeb_mod == schemeb_mod - 1:  # scheme B
                r = tmps.tile([128, 1024], BF, tag="r")
                nc.scalar.activation(r[:], psum_ap, AF.Relu)
                nc.vector.scalar_tensor_tensor(dst_ap, e[:], 1.0, r[:], OP.min, OP.add)
            else:  # scheme A
                t = tmps.tile([128, 1024], BF, tag="t")
                nc.vector.tensor_scalar_min(t[:], e[:], 1.0)
                nc.vector.scalar_tensor_tensor(dst_ap, psum_ap, 0.0, t[:], OP.max, OP.add)

        def body(_iv=None):
            state["span"] = 0
            state["out"] = 0
            for b in range(NB):
                # ---- load x for this batch in column blocks (compute can
                # start as soon as the first cc0/cc1 block pair lands) ----
                X = [xp.tile([128, HW], F32R, tag="x", name=f"x{b}_{cc}") for cc in range(2)]
                for blk in range(4):
                    cs = slice(blk * 1024, (blk + 1) * 1024)
                    for cc in range(2):
                        nc.sync.dma_start(
                            out=X[cc][:, cs],
                            in_=x_d.ap()[b, cc * 128:(cc + 1) * 128, cs],
                        )
                if b == 0:
                    # stage-A/out-proj weights: needed only after stage B, so
                    # they queue behind the first batch's x blocks
                    nc.sync.dma_start(
                        out=w_all[:, 2:4],
                        in_=w_d.ap()[2:4].rearrange("w cc p b -> p w cc b"),
                    )

                # ---- stage B: phi(q^T), phi(v^T), transposed layout [n, o] ----
                # pqv_sb[:, nchunk, 0:256] = phi_qT, [:, nchunk, 256:512] = phi_vT
                pqv_sb = pqvp.tile([128, 32, 512], BF, tag="pqv")
                for i in range(16):
                    ps = psmm.tile([128, 2, 512], FP, tag="mm")
                    for j in range(2):
                        nk = i * 2 + j
                        for (lo, wname) in ((0, "wqt"), (256, "wvt")):
                            for cc in range(2):
                                nc.tensor.matmul(
                                    ps[:, j, lo:lo + 256],
                                    X[cc][:, nk * 128:(nk + 1) * 128],
                                    w_sb[(wname, cc)],
                                    start=(cc == 0),
                                    stop=(cc == 1),
                                )
                    phi_span(flat2(ps[:]), flat2(pqv_sb[:, i * 2:(i + 1) * 2, :]))

                # ---- stage A: phi_k = phi(Wk x), natural layout [o, n] ----
                phik = []
                for m in range(2):
                    pk = phikp.tile([128, HW], BF, tag="phik")
                    phik.append(pk)
                    for i in range(4):
                        ps = psmm.tile([128, 1024], FP, tag="mm")
                        for j in range(2):
                            n0 = (i * 2 + j) * 512
                            for cc in range(2):
                                nc.tensor.matmul(
                                    ps[:, j * 512:(j + 1) * 512],
                                    w_sb[("wkt", cc)][:, m * 128:(m + 1) * 128],
                                    X[cc][:, n0:n0 + 512],
                                    start=(cc == 0),
                                    stop=(cc == 1),
                                )
                        phi_span(ps[:], pk[:, i * 1024:(i + 1) * 1024])

                # ---- stage C: qv[c, d] = sum_n phi_qT[n, c] phi_vT[n, d] ----
                # NOTE: the two cc accumulation chains are interleaved, and
                # matmul start=True clears the whole PSUM *bank*'s has_written
                # bits -- so each chain must live in its own bank.  [128,2,512]
                # spans 2 banks; chain cc writes [:, cc, 0:256] (bank cc).
                qv_ps = psacc.tile([128, 2, 512], FP, tag="mm" if mm_bufs >= 4 else "acc")
                for i in range(32):
                    for cc in range(2):
                        nc.tensor.matmul(
                            qv_ps[:, cc, 0:256],
                            pqv_sb[:, i, cc * 128:(cc + 1) * 128],
                            pqv_sb[:, i, 256:512],
                            start=(i == 0),
                            stop=(i == 31),
                        )
                qv_sb = smalls.tile([128, 2, 256], F32R, tag="qv_sb")
                nc.scalar.activation(qv_sb[:], qv_ps[:, :, 0:256], AF.Copy)

                # ---- stage C2: W2^T[d, o] = sum_c qv[c, d] WoT[c, o] ----
                # dd groups are sequential (not interleaved), one bank is fine.
                w2_ps = psacc.tile([128, 2, 256], FP, tag="mm" if mm_bufs >= 4 else "acc")
                for dd in range(2):
                    for cc in range(2):
                        nc.tensor.matmul(
                            w2_ps[:, dd, :],
                            qv_sb[:, cc, dd * 128:(dd + 1) * 128],
                            w_sb[("wot", cc)][:],
                            start=(cc == 0),
                            stop=(cc == 1),
                        )
                w2_sb = smalls.tile([128, 2, 256], BF, tag="w2_sb")
                nc.scalar.activation(flat2(w2_sb[:]), flat2(w2_ps[:]), AF.Copy)

                # ---- stage D: out[o, n] = sum_d W2[o, d] phi_k[d, n] + bo ----
                for m in range(2):
                    for i in range(4):
                        ps = psmm.tile([128, 1024], FP, tag="mm")
                        for j in range(2):
                            n0 = (i * 2 + j) * 512
                            for dd in range(2):
                                nc.tensor.matmul(
                                    ps[:, j * 512:(j + 1) * 512],
                                    w2_sb[:, dd, m * 128:(m + 1) * 128],
                                    phik[dd][:, n0:n0 + 512],
                                    start=(dd == 0),
                                    stop=(dd == 1),
                                )
                        o_sb = outp.tile([128, 1024], FP, tag="osb")
                        if state["out"] % 2 == 0:
                            nc.scalar.activation(
                                o_sb[:], ps[:], AF.Identity, bias=bo_sb[:, m:m + 1]
                            )
                        else:
                            nc.vector.tensor_scalar_add(o_sb[:], ps[:], bo_sb[:, m:m + 1])
                        state["out"] += 1
                        nc.sync.dma_start(
                            out=out_d.ap()[b, m * 128:(m + 1) * 128, i * 1024:(i + 1) * 1024],
                            in_=o_sb[:],
                        )

        if repeat == 1:
            body()
        else:
            with tc.For_i(0, repeat, 1) as iv:
                body(iv)

    nc.compile()
    return nc


_nc_cache = {}


def _get_nc(repeat: int = 1):
    if repeat not in _nc_cache:
        _nc_cache[repeat] = build_kernel(repeat)
    return _nc_cache[repeat]


def make_in_maps(x, Wq, Wk, Wv, Wo, bo):
    x = np.ascontiguousarray(np.asarray(x, dtype=np.float32).reshape(B, C, HW))
    wall = np.stack(
        [np.asarray(w, dtype=np.float32).T.reshape(2, 128, C) for w in (Wq, Wv, Wk, Wo)]
    )
    wall = np.ascontiguousarray(wall)
    bo2 = np.ascontiguousarray(np.asarray(bo, dtype=np.float32).reshape(C, 1))
    return [
        {"x": x[i * NB:(i + 1) * NB], "wall": wall, "bo": bo2}
        for i in range(NCORES)
    ]


def kernel(x, Wq, Wk, Wv, Wo, bo):
    nc = _get_nc(repeat=1)
    in_maps = make_in_maps(x, Wq, Wk, Wv, Wo, bo)
    res = bass_utils.run_bass_kernel_spmd(nc, in_maps, core_ids=list(range(NCORES)))
    out = np.concatenate([res.results[i]["out"] for i in range(NCORES)], axis=0)
    return np.ascontiguousarray(out.reshape(B, C, H, W).astype(np.float32))


# revision 21
# speedup vs baseline: 1.2174x; 1.2174x over previous
"""Trainium2 Bass kernel for ConvolutionalAttention2D (linear attention with 1x1 convs).

Reference computation (per batch b):
    q = Wq x ; k = Wk x ; v = Wv x          (1x1 convs == channel matmuls)
    phi(t) = elu(t) + 1
    qv = phi(q) @ phi(v)^T                  ([C, C] context matrix, contract over pixels)
    out = Wo (qv @ phi(k)) + bo

Kernel strategy (8 NeuronCores, data-parallel over batch B=16 -> 2 batches/core):
  - Weights replicated, passed pre-transposed from host.
  - Algebraic refactor: Wo (qv @ phi_k) == (Wo qv) @ phi_k; Wo qv is a tiny
    [C, C] product, saving a full [C, HW] projection matmul per batch.
  - phi(t) = min(exp(t), max(t+1, 1)) computed with 1 ACT pass (Exp) and 2 DVE
    ops (or 2 ACT + 1 DVE, mixed to balance engine load).
  - Projections run on the PE in float32r (full fp32 data, 1 cycle/row);
    attention matmuls in bf16 (phi outputs).
"""

from contextlib import ExitStack

import numpy as np

import concourse.bacc as bacc
import concourse.tile as tile
from concourse import mybir
from concourse import bass_utils

B, C, H, W = 16, 256, 64, 64
HW = H * W
NCORES = 8
NB = B // NCORES  # batches per core

FP = mybir.dt.float32
BF = mybir.dt.bfloat16
F32R = mybir.dt.float32r
AF = mybir.ActivationFunctionType
OP = mybir.AluOpType


def flat2(ap):
    return ap.rearrange("p a b -> p (a b)")


def build_kernel(repeat: int = 1, xp_bufs=3, phikp_bufs=2, pqvp_bufs=1, mm_bufs=3,
                 tmps_bufs=4, outp_bufs=6, schemeb_mod=3, out_act_mod=2):
    """Build the per-core Bass program. `repeat` wraps the whole body in a
    dynamic For_i loop (used only for wall-clock timing runs)."""
    nc = bacc.Bacc("TRN2", target_bir_lowering=False, debug=False)

    x_d = nc.dram_tensor("x", [NB, C, HW], F32R, kind="ExternalInput")
    # all four transposed weights in one tensor: [w, cc, 128, C]
    w_d = nc.dram_tensor("wall", [4, 2, 128, C], F32R, kind="ExternalInput")
    bo_d = nc.dram_tensor("bo", [C, 1], FP, kind="ExternalInput")
    out_d = nc.dram_tensor("out", [NB, C, HW], FP, kind="ExternalOutput")

    with tile.TileContext(nc) as tc, ExitStack() as ctx:
        singles = ctx.enter_context(tc.tile_pool(name="singles", bufs=1))
        xp = ctx.enter_context(tc.tile_pool(name="xp", bufs=xp_bufs))
        phikp = ctx.enter_context(tc.tile_pool(name="phikp", bufs=phikp_bufs))
        pqvp = ctx.enter_context(tc.tile_pool(name="pqvp", bufs=pqvp_bufs))
        tmps = ctx.enter_context(tc.tile_pool(name="tmps", bufs=tmps_bufs))
        smalls = ctx.enter_context(tc.tile_pool(name="smalls", bufs=2))
        outp = ctx.enter_context(tc.tile_pool(name="outp", bufs=outp_bufs))
        psmm = ctx.enter_context(tc.tile_pool(name="psmm", bufs=mm_bufs, space="PSUM"))
        psacc = psmm if mm_bufs >= 4 else ctx.enter_context(tc.tile_pool(name="psacc", bufs=1, space="PSUM"))

        # ---- weights (loaded once, replicated; stage-B weights first) ----
        w_all = singles.tile([128, 4, 2, C], F32R, tag="wall")
        nc.sync.dma_start(
            out=w_all[:, 0:2],
            in_=w_d.ap()[0:2].rearrange("w cc p b -> p w cc b"),
        )
        w_sb = {}
        for wi, name in enumerate(("wqt", "wvt", "wkt", "wot")):
            for cc in range(2):
                w_sb[(name, cc)] = w_all[:, wi, cc, :]
        bo_sb = singles.tile([128, 2], FP, tag="bo")
        for m in range(2):
            nc.sync.dma_start(
                out=bo_sb[:, m:m + 1], in_=bo_d.ap()[m * 128:(m + 1) * 128, :]
            )

        state = {"span": 0, "out": 0}

        def phi_span(psum_ap, dst_ap):
            """dst = phi(psum) = min(exp(x), max(x+1, 1)), bf16 out.

            Scheme A (1 ACT + 2 DVE): e=Exp(x); t=min(e,1); dst=(x max 0)+t
            Scheme B (2 ACT + 1 DVE): e=Exp(x); r=Relu(x); dst=(e min 1)+r
            Mixed by span index to balance ACT vs DVE load.
            """
            i = state["span"]
            state["span"] += 1
            e = tmps.tile([128, 1024], BF, tag="e")
            nc.scalar.activation(e[:], psum_ap, AF.Exp)
            if i % schemeb_mod == schemeb_mod - 1:  # scheme B
                r = tmps.tile([128, 1024], BF, tag="r")
                nc.scalar.activation(r[:], psum_ap, AF.Relu)
                nc.vector.scalar_tensor_tensor(dst_ap, e[:], 1.0, r[:], OP.min, OP.add)
            else:  # scheme A
                t = tmps.tile([128, 1024], BF, tag="t")
                nc.vector.tensor_scalar_min(t[:], e[:], 1.0)
                nc.vector.scalar_tensor_tensor(dst_ap, psum_ap, 0.0, t[:], OP.max, OP.add)

        def body(_iv=None):
            state["span"] = 0
            state["out"] = 0
            for b in range(NB):
                # ---- load x for this batch in column blocks (compute can
                # start as soon as the first cc0/cc1 block pair lands) ----
                X = [xp.tile([128, HW], F32R, tag="x", name=f"x{b}_{cc}") for cc in range(2)]
                xblocks = [(0, 512), (512, 512), (1024, 1024), (2048, 1024), (3072, 1024)]
                for (c0, cw) in xblocks:
                    cs = slice(c0, c0 + cw)
                    for cc in range(2):
                        nc.sync.dma_start(
                            out=X[cc][:, cs],
                            in_=x_d.ap()[b, cc * 128:(cc + 1) * 128, cs],
                        )
                if b == 0:
                    # stage-A/out-proj weights: needed only after stage B, so
                    # they queue behind the first batch's x blocks
                    nc.sync.dma_start(
                        out=w_all[:, 2:4],
                        in_=w_d.ap()[2:4].rearrange("w cc p b -> p w cc b"),
                    )

                # ---- stage B: phi(q^T), phi(v^T), transposed layout [n, o] ----
                # pqv_sb[:, nchunk, 0:256] = phi_qT, [:, nchunk, 256:512] = phi_vT
                pqv_sb = pqvp.tile([128, 32, 512], BF, tag="pqv")
                for i in range(16):
                    ps = psmm.tile([128, 2, 512], FP, tag="mm")
                    for j in range(2):
                        nk = i * 2 + j
                        for (lo, wname) in ((0, "wqt"), (256, "wvt")):
                            for cc in range(2):
                                nc.tensor.matmul(
                                    ps[:, j, lo:lo + 256],
                                    X[cc][:, nk * 128:(nk + 1) * 128],
                                    w_sb[(wname, cc)],
                                    start=(cc == 0),
                                    stop=(cc == 1),
                                )
                    phi_span(flat2(ps[:]), flat2(pqv_sb[:, i * 2:(i + 1) * 2, :]))

                # ---- stage A: phi_k = phi(Wk x), natural layout [o, n] ----
                phik = []
                for m in range(2):
                    pk = phikp.tile([128, HW], BF, tag="phik")
                    phik.append(pk)
                    for i in range(4):
                        ps = psmm.tile([128, 1024], FP, tag="mm")
                        for j in range(2):
                            n0 = (i * 2 + j) * 512
                            for cc in range(2):
                                nc.tensor.matmul(
                                    ps[:, j * 512:(j + 1) * 512],
                                    w_sb[("wkt", cc)][:, m * 128:(m + 1) * 128],
                                    X[cc][:, n0:n0 + 512],
                                    start=(cc == 0),
                                    stop=(cc == 1),
                                )
                        phi_span(ps[:], pk[:, i * 1024:(i + 1) * 1024])

                # ---- stage C: qv[c, d] = sum_n phi_qT[n, c] phi_vT[n, d] ----
                # NOTE: the two cc accumulation chains are interleaved, and
                # matmul start=True clears the whole PSUM *bank*'s has_written
                # bits -- so each chain must live in its own bank.  [128,2,512]
                # spans 2 banks; chain cc writes [:, cc, 0:256] (bank cc).
                qv_ps = psacc.tile([128, 2, 512], FP, tag="mm" if mm_bufs >= 4 else "acc")
                for i in range(32):
                    for cc in range(2):
                        nc.tensor.matmul(
                            qv_ps[:, cc, 0:256],
                            pqv_sb[:, i, cc * 128:(cc + 1) * 128],
                            pqv_sb[:, i, 256:512],
                            start=(i == 0),
                            stop=(i == 31),
                        )
                qv_sb = smalls.tile([128, 2, 256], F32R, tag="qv_sb")
                nc.scalar.activation(qv_sb[:], qv_ps[:, :, 0:256], AF.Copy)

                # ---- stage C2: W2^T[d, o] = sum_c qv[c, d] WoT[c, o] ----
                # dd groups are sequential (not interleaved), one bank is fine.
                w2_ps = psacc.tile([128, 2, 256], FP, tag="mm" if mm_bufs >= 4 else "acc")
                for dd in range(2):
                    for cc in range(2):
                        nc.tensor.matmul(
                            w2_ps[:, dd, :],
                            qv_sb[:, cc, dd * 128:(dd + 1) * 128],
                            w_sb[("wot", cc)][:],
                            start=(cc == 0),
                            stop=(cc == 1),
                        )
                w2_sb = smalls.tile([128, 2, 256], BF, tag="w2_sb")
                nc.scalar.activation(flat2(w2_sb[:]), flat2(w2_ps[:]), AF.Copy)

                # ---- stage D: out[o, n] = sum_d W2[o, d] phi_k[d, n] + bo ----
                for m in range(2):
                    for i in range(4):
                        ps = psmm.tile([128, 1024], FP, tag="mm")
                        for j in range(2):
                            n0 = (i * 2 + j) * 512
                            for dd in range(2):
                                nc.tensor.matmul(
                                    ps[:, j * 512:(j + 1) * 512],
                                    w2_sb[:, dd, m * 128:(m + 1) * 128],
                                    phik[dd][:, n0:n0 + 512],
                                    start=(dd == 0),
                                    stop=(dd == 1),
                                )
                        o_sb = outp.tile([128, 1024], FP, tag="osb")
                        if state["out"] % out_act_mod == 0:
                            nc.scalar.activation(
                                o_sb[:], ps[:], AF.Identity, bias=bo_sb[:, m:m + 1]
                            )
                        else:
                            nc.vector.tensor_scalar_add(o_sb[:], ps[:], bo_sb[:, m:m + 1])
                        state["out"] += 1
                        nc.sync.dma_start(
                            out=out_d.ap()[b, m * 128:(m + 1) * 128, i * 1024:(i + 1) * 1024],
                            in_=o_sb[:],
                        )

        if repeat == 1:
            body()
        else:
            with tc.For_i(0, repeat, 1) as iv:
                body(iv)

    nc.compile()
    return nc


_nc_cache = {}


def _get_nc(repeat: int = 1):
    if repeat not in _nc_cache:
        _nc_cache[repeat] = build_kernel(repeat)
    return _nc_cache[repeat]


def make_in_maps(x, Wq, Wk, Wv, Wo, bo):
    x = np.ascontiguousarray(np.asarray(x, dtype=np.float32).reshape(B, C, HW))
    wall = np.stack(
        [np.asarray(w, dtype=np.float32).T.reshape(2, 128, C) for w in (Wq, Wv, Wk, Wo)]
    )
    wall = np.ascontiguousarray(wall)
    bo2 = np.ascontiguousarray(np.asarray(bo, dtype=np.float32).reshape(C, 1))
    return [
        {"x": x[i * NB:(i + 1) * NB], "wall": wall, "bo": bo2}
        for i in range(NCORES)
    ]


def kernel(x, Wq, Wk, Wv, Wo, bo):
    nc = _get_nc(repeat=1)
    in_maps = make_in_maps(x, Wq, Wk, Wv, Wo, bo)
    res = bass_utils.run_bass_kernel_spmd(nc, in_maps, core_ids=list(range(NCORES)))
    out = np.concatenate([res.results[i]["out"] for i in range(NCORES)], axis=0)
    return np.ascontiguousarray(out.reshape(B, C, H, W).astype(np.float32))
